# revision 1
# baseline (speedup 1.0000x reference)
"""Trainium2 Bass kernel for the binarized CNN:
conv3x3(sign weights) -> BN -> ternary hardtanh -> maxpool4 -> linear(sign weights)

Strategy (data parallel, 8 cores x 512 samples):
  - BN + ternary + maxpool commute (bn_gamma > 0): pool the raw conv and
    threshold the pooled max. Weights are scaled by q_c = fp16(1/(Thi-Tlo))
    so the thresholds become constants: hi = 0 (exact: two fp16 ones-rows
    in the matmul carry -Thi*q split to ~2^-22), lo = -1 (~2^-12 off,
    safe in the far tail of max-of-16).
  - Conv is ONE fp16 matmul stream per (batch-tile, row): stationary =
    host-built im2col patch [116, 128], moving = +-q_c pattern [116, 1152]
    in phase-major column order (w-pool = max over 4 contiguous blocks).
    fp16 streams at 1 cycle/row on the PE (fp32 is 2) and fp16 products
    accumulate exactly in fp32 PSUM; verified 0 ternary flips end-to-end.
  - PSUM: z-main [128,1024]x3 bufs + shared tail tile keeps the PE 3 rows
    ahead of pooling; matmul chunks stay 512-col bank-aligned.
  - Pooling alternates two paths to balance Scalar and Vector: 'S' pairs
    (scalar converts both halves to fp16 SBUF, vector h-maxes at DVE
    16-bit rate) and 'H' pairs (scalar converts h0 only, vector TTs h1
    straight from PSUM against it). GPSIMD cannot touch PSUM and its
    tensor ops are Q7-slow, so it only runs DMA issue duties.
  - Ternary: lt = (M < -1); t = (M > 0) - lt via scalar_tensor_tensor.
  - FC: PE-transposes the fp16 ternary tile (fp16 PSUM transpose is legal
    on TRN2), one wide copy per batch-tile, 7 accumulating fp16 matmuls
    per batch-tile column slice, bias via activation, PE-transpose out.
"""

import numpy as np
from contextlib import ExitStack

import concourse.bass as bass
import concourse.tile as tile
from concourse import bacc, mybir
from concourse.bass_utils import run_bass_kernel_spmd

F32 = mybir.dt.float32
F16 = mybir.dt.float16
ALU = mybir.AluOpType
ACT = mybir.ActivationFunctionType

NCORES = 8
BFULL = 4096
B = BFULL // NCORES          # 512 per core
P = 128
BT = B // P                  # 4 batch tiles
H, W = 14, 38
HO, WO = 12, 36
C = 32
KP = 116                     # 114 patch rows + 2 threshold-ones rows
NF = C * WO                  # 1152 conv outputs per (b, h)
FP = 864                     # pooled features per sample (3 h3 x 288)
EPS = 1e-5
NOUT = 10
FCJ = (128, 128, 128, 128, 128, 128, 96)   # 864 split into 7 K-chunks

# pooling path per pair (6 pairs per bt: (h3, half)):
#   'S': scalar converts both PSUM halves to fp16 SBUF, vector h-maxes them
#   'H': scalar converts h0 only; vector TTs h1 straight from PSUM against
#        the converted h0 (TT allows one PSUM operand)
PAIR_ENG = [
    ['S', 'H', 'S', 'H', 'S', 'H'],
    ['H', 'S', 'H', 'S', 'H', 'S'],
    ['S', 'H', 'S', 'H', 'S', 'H'],
    ['H', 'S', 'H', 'S', 'H', 'S'],
]


def _host_prep(conv_w, conv_b, bn_gamma, bn_beta, bn_mean, bn_var, fc_w, fc_b):
    inv = (bn_gamma.astype(np.float64) / np.sqrt(bn_var.astype(np.float64) + EPS))
    thi = (0.5 - bn_beta) / inv + bn_mean - conv_b       # [32] float64
    tlo = (-0.5 - bn_beta) / inv + bn_mean - conv_b
    # fold 1/(thi-tlo) into the weights so the pooled ternary thresholds
    # become the constants 0 (hi, exact: ones-rows carry the split shift)
    # and -1 (lo, ~2^-12 off which is safe in the far tail)
    q16 = np.float16(1.0 / (thi - tlo))
    u64 = thi * q16.astype(np.float64)
    u_h = np.float16(u64)
    u_l = np.float16(u64 - u_h.astype(np.float64))
    sw = np.sign(conv_w[:, 0]).astype(np.float32)        # [32,3,3]

    # phase-major columns: col = (w%4)*288 + c*9 + (w//4) so the w-pool is
    # a max over 4 contiguous 288 blocks (fast DVE tensor_tensor)
    wt = np.zeros((KP, NF), np.float16)
    for c in range(C):
        for w in range(WO):
            n = (w % 4) * 288 + c * 9 + (w // 4)
            for i in range(3):
                for j in range(3):
                    wt[i * W + w + j, n] = np.float16(sw[c, i, j] * q16[c])
            wt[114, n] = -u_h[c]
            wt[115, n] = -u_l[c]

    # pooled feature order (ours): f = h3*288 + c*9 + g ; reference flatten:
    # f_ref = c*27 + h3*9 + g
    sf = np.sign(fc_w).astype(np.float16)                # [10, 864]
    sfc = np.zeros((P, 7 * NOUT), np.float16)
    for j, kj in enumerate(FCJ):
        for r in range(kj):
            f = j * 128 + r
            h3, rem = f // 288, f % 288
            c, g = rem // 9, rem % 9
            fref = c * 27 + h3 * 9 + g
            sfc[r, j * NOUT:(j + 1) * NOUT] = sf[:, fref]

    fcb = fc_b.astype(np.float32).reshape(NOUT, 1)
    eye16 = np.eye(P, dtype=np.float16)
    eye32 = np.eye(P, dtype=np.float32)
    return wt, sfc, fcb, eye16, eye32


def _host_im2col(x):
    """x [4096, 532] fp32 -> per (core, bt) im tiles [116, 1536] fp16."""
    xh = np.float16(x)                                   # [4096, 532]
    win = np.lib.stride_tricks.as_strided(
        xh, shape=(BFULL, HO, 114),
        strides=(xh.strides[0], W * 2, 2))
    ims = []
    for core in range(NCORES):
        row = []
        for bt in range(BT):
            s = core * B + bt * P
            blk = np.transpose(win[s:s + P], (2, 1, 0))  # [114, 12, 128]
            im = np.empty((KP, HO * P), np.float16)
            im[0:114] = blk.reshape(114, HO * P)
            im[114:116] = 1.0
            row.append(np.ascontiguousarray(im))
        ims.append(row)
    return ims


def _build():
    nc = bacc.Bacc("TRN2", target_bir_lowering=False, debug=False,
                   num_devices=NCORES)
    im_d = [nc.dram_tensor(f"im{bt}", [KP, HO * P], F16,
                           kind="ExternalInput").ap() for bt in range(BT)]
    wt_d = nc.dram_tensor("wt", [KP, NF], F16, kind="ExternalInput").ap()
    sfc_d = nc.dram_tensor("sfc", [P, 7 * NOUT], F16, kind="ExternalInput").ap()
    fcb_d = nc.dram_tensor("fcb", [NOUT, 1], F32, kind="ExternalInput").ap()
    id16_d = nc.dram_tensor("id16", [P, P], F16, kind="ExternalInput").ap()
    id32_d = nc.dram_tensor("id32", [P, P], F32, kind="ExternalInput").ap()
    out_d = nc.dram_tensor("out", [B, NOUT], F32, kind="ExternalOutput").ap()

    with tile.TileContext(nc) as tc, ExitStack() as ctx:
        const = ctx.enter_context(tc.tile_pool(name="const", bufs=1))
        imp = ctx.enter_context(tc.tile_pool(name="imp", bufs=1))
        zsp = ctx.enter_context(tc.tile_pool(name="zsp", bufs=4))
        msp = ctx.enter_context(tc.tile_pool(name="msp", bufs=6))
        upp = ctx.enter_context(tc.tile_pool(name="upp", bufs=2))
        mtp = ctx.enter_context(tc.tile_pool(name="mtp", bufs=2))
        ttp = ctx.enter_context(tc.tile_pool(name="ttp", bufs=1))

        wt = const.tile([KP, NF], F16, tag="wt")
        nc.scalar.dma_start(wt[:], wt_d)
        sfc = const.tile([P, 7 * NOUT], F16, tag="sfc")
        nc.gpsimd.dma_start(sfc[:], sfc_d)
        fcb = const.tile([NOUT, 1], F32, tag="fcb")
        nc.gpsimd.dma_start(fcb[:], fcb_d)
        id16 = const.tile([P, P], F16, tag="id16")
        nc.scalar.dma_start(id16[:], id16_d)
        id32 = const.tile([P, P], F32, tag="id32")
        nc.gpsimd.dma_start(id32[:], id32_d)

        ims = []
        for bt in range(BT):
            imt = imp.tile([KP, HO * P], F16, tag=f"im{bt}", name=f"im{bt}")
            nc.sync.dma_start(imt[:], im_d[bt])
            ims.append(imt)

        # bt-major transposed ternary: bt block = [j(7) x 128 batch cols]
        tTall = ttp.tile([P, BT * 7 * P], F16, tag="tTall")

        with tc.tile_pool(name="zp", bufs=3, space="PSUM") as zp, \
             tc.tile_pool(name="ztp", bufs=1, space="PSUM") as ztp, \
             tc.tile_pool(name="pop", bufs=1, space="PSUM") as pop:
            wu = const.tile([KP, 512], F16, tag="wu")
            nc.vector.memset(wu[:], 1.0)
            zw = zp.tile([P, 1024], F32, tag="z", name="warm")
            for r in range(10):
                nc.tensor.matmul(zw[:, 0:512], lhsT=wu[:, 0:P],
                                 rhs=wu[:], start=True, stop=True)
            for bt in range(BT):
                up = upp.tile([P, 6 * 288], F16, tag="up", name=f"up{bt}")
                for h3 in range(3):
                    for half in range(2):
                        pi = h3 * 2 + half
                        zh = []
                        zt = ztp.tile([P, 256], F32, tag="zt",
                                      name=f"zt{bt}_{pi}")
                        for hh in range(2):
                            h = 4 * h3 + 2 * half + hh
                            k = h * P
                            z = zp.tile([P, 1024], F32, tag="z",
                                        name=f"z{bt}_{h}")
                            for n0, n1 in ((0, 512), (512, 1024)):
                                nc.tensor.matmul(
                                    z[:, n0:n1],
                                    lhsT=ims[bt][:, k:k + P],
                                    rhs=wt[:, n0:n1],
                                    start=True, stop=True)
                            nc.tensor.matmul(
                                zt[:, hh * 128:(hh + 1) * 128],
                                lhsT=ims[bt][:, k:k + P],
                                rhs=wt[:, 1024:NF],
                                start=True, stop=True)
                            zh.append(z)
                        # pair-pool: u[cg] = max over (hh, ww window of 4)
                        # up slot: half-major [half*864 + h3*288 + cg]
                        uslot = up[:, half * 864 + h3 * 288:
                                   half * 864 + h3 * 288 + 288]
                        eng = PAIR_ENG[bt][pi]
                        m = msp.tile([P, NF], F16, tag="m",
                                     name=f"m{bt}_{pi}")
                        if eng == 'S':
                            zs = zsp.tile([P, 2304], F16, tag="zs",
                                          name=f"zs{bt}_{pi}")
                            nc.scalar.copy(zs[:, 0:1024], zh[0][:, 0:1024])
                            nc.scalar.copy(zs[:, 1024:2048], zh[1][:, 0:1024])
                            nc.scalar.copy(zs[:, 2048:2304], zt[:, 0:256])
                            nc.vector.tensor_max(m[:, 0:1024], zs[:, 0:1024],
                                                 zs[:, 1024:2048])
                            nc.vector.tensor_max(m[:, 1024:NF],
                                                 zs[:, 2048:2176],
                                                 zs[:, 2176:2304])
                        else:  # H: scalar converts h0, vector TTs psum h1
                            m0 = msp.tile([P, NF], F16, tag="m0",
                                          name=f"m0{bt}_{pi}")
                            nc.scalar.copy(m0[:, 0:1024], zh[0][:, 0:1024])
                            nc.scalar.copy(m0[:, 1024:NF], zt[:, 0:128])
                            nc.vector.tensor_max(m[:, 0:1024],
                                                 zh[1][:, 0:1024],
                                                 m0[:, 0:1024])
                            nc.vector.tensor_max(m[:, 1024:NF],
                                                 zt[:, 128:256],
                                                 m0[:, 1024:NF])
                        # w-pool: phase-major tree over 4 contiguous blocks
                        n2 = msp.tile([P, 576], F16, tag="n2",
                                      name=f"n2{bt}_{pi}")
                        nc.vector.tensor_max(n2[:], m[:, 0:576], m[:, 576:NF])
                        nc.vector.tensor_max(uslot, n2[:, 0:288],
                                             n2[:, 288:576])

                # finalize bt: M = max over halves; L; t
                mt = mtp.tile([P, FP], F16, tag="mt", name=f"mt{bt}")
                nc.vector.tensor_max(mt[:], up[:, 0:FP], up[:, FP:2 * FP])
                lt = mtp.tile([P, FP], F16, tag="lt", name=f"lt{bt}")
                nc.vector.tensor_scalar(lt[:], mt[:], -1.0, None, ALU.is_lt)
                t_ = mtp.tile([P, FP], F16, tag="t_", name=f"t{bt}")
                nc.vector.scalar_tensor_tensor(
                    t_[:], mt[:], 0.0, lt[:], ALU.is_gt, ALU.subtract)

                # transpose t via PE (fp16 psum out): 7 chunks into one
                # wide psum tile, then a single copy into tTall's bt block
                po = pop.tile([P, 7 * P], F16, tag="po", name=f"po{bt}")
                for j, kj in enumerate(FCJ):
                    nc.tensor.transpose(po[0:kj, j * P:(j + 1) * P],
                                        t_[:, j * 128:j * 128 + kj],
                                        id16[:])
                if bt % 2 == 0:
                    nc.vector.tensor_copy(
                        tTall[:, bt * 7 * P:(bt + 1) * 7 * P], po[:, :])
                else:
                    nc.scalar.copy(
                        tTall[:, bt * 7 * P:(bt + 1) * 7 * P], po[:, :])

        # FC: out.T[10, 512] accumulated over 7 K-chunks, per bt column slice
        with tc.tile_pool(name="fcp", bufs=1, space="PSUM") as fcp, \
             tc.tile_pool(name="otp", bufs=2, space="PSUM") as otp:
            acc = fcp.tile([NOUT, B], F32, tag="acc")
            for bt in range(BT):
                for j, kj in enumerate(FCJ):
                    nc.tensor.matmul(
                        acc[:, bt * P:(bt + 1) * P],
                        lhsT=sfc[0:kj, j * NOUT:(j + 1) * NOUT],
                        rhs=tTall[0:kj, bt * 7 * P + j * P:
                                  bt * 7 * P + (j + 1) * P],
                        start=(j == 0), stop=(j == 6))

            ob = const.tile([P, B], F32, tag="ob")
            nc.vector.memset(ob[:], 0.0)
            nc.scalar.activation(ob[0:NOUT, :], acc[:],
                                 ACT.Identity,
                                 bias=fcb[0:NOUT, 0:1], scale=1.0)

            for bt in range(BT):
                po2 = otp.tile([P, P], F32, tag="po2", name=f"po2{bt}")
                nc.tensor.transpose(po2[:, :], ob[:, bt * P:(bt + 1) * P],
                                    id32[:])
                os_ = const.tile([P, NOUT], F32, tag=f"os{bt}", name=f"os{bt}")
                nc.scalar.copy(os_[:], po2[0:P, 0:NOUT])
                nc.sync.dma_start(out_d[bt * P:(bt + 1) * P, :], os_[:])

    nc.compile()
    return nc


_NC_CACHE = None


def kernel(x, conv_w, conv_b, bn_gamma, bn_beta, bn_mean, bn_var, fc_w, fc_b):
    global _NC_CACHE
    x = np.asarray(x, np.float32).reshape(BFULL, H * W)
    wt, sfc, fcb, eye16, eye32 = _host_prep(
        np.asarray(conv_w, np.float32), np.asarray(conv_b, np.float32),
        np.asarray(bn_gamma, np.float32), np.asarray(bn_beta, np.float32),
        np.asarray(bn_mean, np.float32), np.asarray(bn_var, np.float32),
        np.asarray(fc_w, np.float32), np.asarray(fc_b, np.float32))
    ims = _host_im2col(x)

    if _NC_CACHE is None:
        _NC_CACHE = _build()
    nc = _NC_CACHE

    in_maps = [
        dict(wt=wt, sfc=sfc, fcb=fcb, id16=eye16, id32=eye32,
             **{f"im{bt}": ims[i][bt] for bt in range(BT)})
        for i in range(NCORES)
    ]
    res = run_bass_kernel_spmd(nc, in_maps, core_ids=list(range(NCORES)))
    out = np.concatenate([res.results[i]["out"] for i in range(NCORES)], axis=0)
    return out.astype(np.float32)



# revision 2
# speedup vs baseline: 1.0491x; 1.0491x over previous
"""Trainium2 Bass kernel v3 for the binarized CNN:
conv3x3(sign weights) -> BN -> ternary hardtanh -> maxpool4 -> linear(sign weights)

v3 scheduling changes over the v1 baseline (same numerics / host prep):
  - Startup: no PE warmup; wt + im0 chunks ride the sync hardware-DGE
    queue (gpsimd's software DGE is ~40GB/s - too slow for the critical
    path), im1 rides the scalar hardware queue; first conv matmul ~9.5us
    instead of ~17us.
  - The two w-pool levels are batched across each h3's two half-pairs as
    single 3D-strided DVE ops (one [128,2,576] max + one [128,2,288] max
    instead of four 2D ops) - saves ~1us of DVE time per batch tile.
  - FC is interleaved per batch tile: the FC accumulator and the output
    transpose tile rotate through the same PSUM tag (same bank) as the
    transpose tile po, so no separate FC phase / pool-swap drain barrier.
  - Bias via activation into an fp16 row tile, 10x10-identity PE
    transposes (id32 dropped), per-bt output DMA - short serial tail.
"""

import numpy as np
from contextlib import ExitStack

import concourse.bass as bass
import concourse.tile as tile
from concourse import bacc, mybir
from concourse.bass_utils import run_bass_kernel_spmd

F32 = mybir.dt.float32
F16 = mybir.dt.float16
ALU = mybir.AluOpType
ACT = mybir.ActivationFunctionType

NCORES = 8
BFULL = 4096
B = BFULL // NCORES          # 512 per core
P = 128
BT = B // P                  # 4 batch tiles
H, W = 14, 38
HO, WO = 12, 36
C = 32
KP = 116                     # 114 patch rows + 2 threshold-ones rows
NF = C * WO                  # 1152 conv outputs per (b, h)
FP = 864                     # pooled features per sample (3 h3 x 288)
EPS = 1e-5
NOUT = 10
FCJ = (128, 128, 128, 128, 128, 128, 96)   # 864 split into 7 K-chunks

# ---- schedule config ----
# pooling path per pair: 'S' = scalar converts both halves + tail, vector
# maxes fp16; 'H' = scalar converts h0 only, vector maxes h1 from PSUM.
PAIR_ENG = [
    ['S', 'H', 'H', 'H', 'S', 'H'],
    ['S', 'H', 'H', 'H', 'S', 'H'],
    ['S', 'H', 'H', 'H', 'S', 'H'],
    ['S', 'H', 'S', 'H', 'S', 'H'],
]
COPY_ENG = ['S', 'S', 'S', 'S']   # po->tTall copy engine per bt


def _host_prep(conv_w, conv_b, bn_gamma, bn_beta, bn_mean, bn_var, fc_w, fc_b):
    inv = (bn_gamma.astype(np.float64) / np.sqrt(bn_var.astype(np.float64) + EPS))
    thi = (0.5 - bn_beta) / inv + bn_mean - conv_b       # [32] float64
    tlo = (-0.5 - bn_beta) / inv + bn_mean - conv_b
    # fold 1/(thi-tlo) into the weights so the pooled ternary thresholds
    # become the constants 0 (hi, exact: ones-rows carry the split shift)
    # and -1 (lo, ~2^-12 off which is safe in the far tail)
    q16 = np.float16(1.0 / (thi - tlo))
    u64 = thi * q16.astype(np.float64)
    u_h = np.float16(u64)
    u_l = np.float16(u64 - u_h.astype(np.float64))
    sw = np.sign(conv_w[:, 0]).astype(np.float32)        # [32,3,3]

    # phase-major columns: col = (w%4)*288 + c*9 + (w//4) so the w-pool is
    # a max over 4 contiguous 288 blocks (fast DVE tensor_tensor)
    wt = np.zeros((KP, NF), np.float16)
    for c in range(C):
        for w in range(WO):
            n = (w % 4) * 288 + c * 9 + (w // 4)
            for i in range(3):
                for j in range(3):
                    wt[i * W + w + j, n] = np.float16(sw[c, i, j] * q16[c])
            wt[114, n] = -u_h[c]
            wt[115, n] = -u_l[c]

    # pooled feature order (ours): f = h3*288 + c*9 + g ; reference flatten:
    # f_ref = c*27 + h3*9 + g
    sf = np.sign(fc_w).astype(np.float16)                # [10, 864]
    sfc = np.zeros((P, 7 * NOUT), np.float16)
    for j, kj in enumerate(FCJ):
        for r in range(kj):
            f = j * 128 + r
            h3, rem = f // 288, f % 288
            c, g = rem // 9, rem % 9
            fref = c * 27 + h3 * 9 + g
            sfc[r, j * NOUT:(j + 1) * NOUT] = sf[:, fref]

    fcb = fc_b.astype(np.float32).reshape(NOUT, 1)
    eye16 = np.eye(P, dtype=np.float16)
    return wt, sfc, fcb, eye16


def _host_im2col(x):
    """x [4096, 532] fp32 -> per (core, bt) im tiles [116, 1536] fp16."""
    xh = np.float16(x)                                   # [4096, 532]
    win = np.lib.stride_tricks.as_strided(
        xh, shape=(BFULL, HO, 114),
        strides=(xh.strides[0], W * 2, 2))
    ims = []
    for core in range(NCORES):
        row = []
        for bt in range(BT):
            s = core * B + bt * P
            blk = np.transpose(win[s:s + P], (2, 1, 0))  # [114, 12, 128]
            im = np.empty((KP, HO * P), np.float16)
            im[0:114] = blk.reshape(114, HO * P)
            im[114:116] = 1.0
            row.append(np.ascontiguousarray(im))
        ims.append(row)
    return ims


def _build():
    nc = bacc.Bacc("TRN2", target_bir_lowering=False, debug=False,
                   num_devices=NCORES)
    im_d = [nc.dram_tensor(f"im{bt}", [KP, HO * P], F16,
                           kind="ExternalInput").ap() for bt in range(BT)]
    wt_d = nc.dram_tensor("wt", [KP, NF], F16, kind="ExternalInput").ap()
    sfc_d = nc.dram_tensor("sfc", [P, 7 * NOUT], F16, kind="ExternalInput").ap()
    fcb_d = nc.dram_tensor("fcb", [NOUT, 1], F32, kind="ExternalInput").ap()
    id16_d = nc.dram_tensor("id16", [P, P], F16, kind="ExternalInput").ap()
    out_d = nc.dram_tensor("out", [B, NOUT], F32, kind="ExternalOutput").ap()

    with tile.TileContext(nc) as tc, ExitStack() as ctx:
        const = ctx.enter_context(tc.tile_pool(name="const", bufs=1))
        imp = ctx.enter_context(tc.tile_pool(name="imp", bufs=1))
        zsp = ctx.enter_context(tc.tile_pool(name="zsp", bufs=4))
        msp = ctx.enter_context(tc.tile_pool(name="msp", bufs=6))
        upp = ctx.enter_context(tc.tile_pool(name="upp", bufs=2))
        mtp = ctx.enter_context(tc.tile_pool(name="mtp", bufs=2))
        ttp = ctx.enter_context(tc.tile_pool(name="ttp", bufs=1))

        wt = const.tile([KP, NF], F16, tag="wt")
        sfc = const.tile([P, 7 * NOUT], F16, tag="sfc")
        fcb = const.tile([NOUT, 1], F32, tag="fcb")
        id16 = const.tile([P, P], F16, tag="id16")
        ob = const.tile([16, B], F16, tag="ob")
        ims = [imp.tile([KP, HO * P], F16, tag=f"im{bt}", name=f"im{bt}")
               for bt in range(BT)]

        # ---- DMA issue schedule: first-conv needs im0's first columns +
        # wt[0:512]; split them across the two hardware-DGE queues (sync +
        # scalar) so they transfer in parallel; small late-needed consts go
        # to the slow gpsimd software queue.
        nc.sync.dma_start(ims[0][:, 0:512], im_d[0][:, 0:512])
        nc.sync.dma_start(wt[:, 512:NF], wt_d[:, 512:NF])
        nc.sync.dma_start(ims[0][:, 512:1024], im_d[0][:, 512:1024])
        nc.sync.dma_start(ims[0][:, 1024:1536], im_d[0][:, 1024:1536])
        nc.sync.dma_start(ims[2][:], im_d[2])
        nc.sync.dma_start(ims[3][:], im_d[3])
        nc.scalar.dma_start(wt[:, 0:512], wt_d[:, 0:512])
        nc.scalar.dma_start(ims[1][:], im_d[1])
        nc.gpsimd.dma_start(sfc[:], sfc_d)
        nc.gpsimd.dma_start(fcb[:], fcb_d)
        nc.gpsimd.dma_start(id16[:], id16_d)

        # bt-major transposed ternary: bt block = [j(7) x 128 batch cols]
        tTall = ttp.tile([P, BT * 7 * P], F16, tag="tTall")

        with tc.tile_pool(name="zp", bufs=3, space="PSUM") as zp, \
             tc.tile_pool(name="ztp", bufs=1, space="PSUM") as ztp, \
             tc.tile_pool(name="pop", bufs=1, space="PSUM") as pop:

            def emit_fc_acc(bt):
                """FC matmuls + bias-activation for bt (acc rotates through
                the po PSUM tag/bank)."""
                accb = pop.tile([NOUT, P], F32, tag="po", name=f"acc{bt}")
                for j, kj in enumerate(FCJ):
                    nc.tensor.matmul(
                        accb[:, :],
                        lhsT=sfc[0:kj, j * NOUT:(j + 1) * NOUT],
                        rhs=tTall[0:kj, bt * 7 * P + j * P:
                                  bt * 7 * P + (j + 1) * P],
                        start=(j == 0), stop=(j == 6))
                nc.scalar.activation(ob[0:NOUT, bt * P:(bt + 1) * P],
                                     accb[:, :], ACT.Identity,
                                     bias=fcb[0:NOUT, 0:1], scale=1.0)

            def emit_fc_out(bt):
                """Transpose the fp16 FC row back to batch-major and DMA."""
                po2 = pop.tile([P, NOUT], F16, tag="po", name=f"po2{bt}")
                nc.tensor.transpose(po2[0:P, 0:NOUT],
                                    ob[0:NOUT, bt * P:(bt + 1) * P],
                                    id16[0:NOUT, 0:NOUT])
                os_ = const.tile([P, NOUT], F32, tag=f"os{bt}", name=f"os{bt}")
                nc.scalar.copy(os_[:], po2[:, 0:NOUT])
                nc.sync.dma_start(out_d[bt * P:(bt + 1) * P, :], os_[:])

            for bt in range(BT):
                # up3[p, half, h3*288+cg] - same layout as a flat half-major
                # [P, 1728] tile, 3D-viewed for the batched w-pool writes
                up3 = upp.tile([P, 2, 3 * 288], F16, tag="up", name=f"up{bt}")
                mt = mtp.tile([P, FP], F16, tag="mt", name=f"mt{bt}")
                lt = mtp.tile([P, FP], F16, tag="lt", name=f"lt{bt}")
                t_ = mtp.tile([P, FP], F16, tag="t_", name=f"t{bt}")
                po = None
                for h3 in range(3):
                    # m2[p, half, 1152]: both half-pairs of this h3 group
                    m2 = msp.tile([P, 2, NF], F16, tag="m2",
                                  name=f"m2{bt}_{h3}")
                    for half in range(2):
                        pi = h3 * 2 + half
                        zh = []
                        zt = ztp.tile([P, 256], F32, tag="zt",
                                      name=f"zt{bt}_{pi}")
                        for hh in range(2):
                            h = 4 * h3 + 2 * half + hh
                            k = h * P
                            z = zp.tile([P, 1024], F32, tag="z",
                                        name=f"z{bt}_{h}")
                            for n0, n1 in ((0, 512), (512, 1024)):
                                nc.tensor.matmul(
                                    z[:, n0:n1],
                                    lhsT=ims[bt][:, k:k + P],
                                    rhs=wt[:, n0:n1],
                                    start=True, stop=True)
                            nc.tensor.matmul(
                                zt[:, hh * 128:(hh + 1) * 128],
                                lhsT=ims[bt][:, k:k + P],
                                rhs=wt[:, 1024:NF],
                                start=True, stop=True)
                            zh.append(z)
                        eng = PAIR_ENG[bt][pi]
                        if eng == 'S':
                            zs = zsp.tile([P, 2304], F16, tag="zs",
                                          name=f"zs{bt}_{pi}")
                            # z0 first (ready earliest), then zt (frees the
                            # single zt bank before the next pair's tail
                            # matmul), then z1
                            nc.scalar.copy(zs[:, 0:1024], zh[0][:, 0:1024])
                            nc.scalar.copy(zs[:, 2048:2304], zt[:, 0:256])
                            nc.scalar.copy(zs[:, 1024:2048], zh[1][:, 0:1024])
                            nc.vector.tensor_max(m2[:, half, 1024:NF],
                                                 zs[:, 2048:2176],
                                                 zs[:, 2176:2304])
                            nc.vector.tensor_max(m2[:, half, 0:1024],
                                                 zs[:, 0:1024],
                                                 zs[:, 1024:2048])
                        else:  # H: scalar converts h0, vector TTs psum h1
                            m0 = msp.tile([P, NF], F16, tag="m0",
                                          name=f"m0{bt}_{pi}")
                            nc.scalar.copy(m0[:, 0:1024], zh[0][:, 0:1024])
                            nc.scalar.copy(m0[:, 1024:NF], zt[:, 0:128])
                            nc.vector.tensor_max(m2[:, half, 1024:NF],
                                                 zt[:, 128:256],
                                                 m0[:, 1024:NF])
                            nc.vector.tensor_max(m2[:, half, 0:1024],
                                                 zh[1][:, 0:1024],
                                                 m0[:, 0:1024])
                        # deferred FC of the previous bt: its tTall copy is
                        # ~2 pairs old by now, so the PE queue won't block.
                        # Split so the po2 transpose (which waits on the
                        # scalar act) sits another pair later in PE order.
                        if bt > 0 and h3 == 0 and half == 1:
                            emit_fc_acc(bt - 1)
                        if bt > 0 and h3 == 1 and half == 0:
                            emit_fc_out(bt - 1)
                    # batched w-pool for both halves of this h3 group:
                    # level 1: [P,2,576], level 2: [P,2,288] into up slots
                    n4 = msp.tile([P, 2, 576], F16, tag="n4",
                                  name=f"n4{bt}_{h3}")
                    nc.vector.tensor_max(n4[:, :, :], m2[:, :, 0:576],
                                         m2[:, :, 576:NF])
                    nc.vector.tensor_max(up3[:, :, h3 * 288:(h3 + 1) * 288],
                                         n4[:, :, 0:288], n4[:, :, 288:576])

                    # bt3: finalize h3 groups early to shorten the tail
                    if bt == BT - 1 and h3 >= 1:
                        if h3 == 1:
                            c0, c1 = 0, 576
                            jlist = [0, 1, 2, 3]
                        else:
                            c0, c1 = 576, 864
                            jlist = [4, 5, 6]
                        _finalize(nc, up3, mt, lt, t_, c0, c1)
                        if po is None:
                            po = pop.tile([P, 7 * P], F16, tag="po",
                                          name=f"po{bt}")
                        for j in jlist:
                            kj = FCJ[j]
                            nc.tensor.transpose(
                                po[0:kj, j * P:(j + 1) * P],
                                t_[:, j * 128:j * 128 + kj], id16[:])
                        ccol = (jlist[0] * P, (jlist[-1] + 1) * P)
                        nc.scalar.copy(
                            tTall[:, bt * 7 * P + ccol[0]:
                                  bt * 7 * P + ccol[1]],
                            po[:, ccol[0]:ccol[1]])

                if bt < BT - 1:
                    _finalize(nc, up3, mt, lt, t_, 0, FP)
                    po = pop.tile([P, 7 * P], F16, tag="po", name=f"po{bt}")
                    for j, kj in enumerate(FCJ):
                        nc.tensor.transpose(po[0:kj, j * P:(j + 1) * P],
                                            t_[:, j * 128:j * 128 + kj],
                                            id16[:])
                    if COPY_ENG[bt] == 'V':
                        nc.vector.tensor_copy(
                            tTall[:, bt * 7 * P:(bt + 1) * 7 * P], po[:, :])
                    else:
                        nc.scalar.copy(
                            tTall[:, bt * 7 * P:(bt + 1) * 7 * P], po[:, :])

            # bt3's FC runs in the tail
            emit_fc_acc(BT - 1)
            emit_fc_out(BT - 1)

    nc.compile()
    return nc


def _finalize(nc, up3, mt, lt, t_, c0, c1):
    """M = max over halves; lt = M < -1; t = (M > 0) - lt, on cols [c0,c1)."""
    nc.vector.tensor_max(mt[:, c0:c1], up3[:, 0, c0:c1], up3[:, 1, c0:c1])
    nc.vector.tensor_scalar(lt[:, c0:c1], mt[:, c0:c1], -1.0, None,
                            ALU.is_lt)
    nc.vector.scalar_tensor_tensor(
        t_[:, c0:c1], mt[:, c0:c1], 0.0, lt[:, c0:c1],
        ALU.is_gt, ALU.subtract)


_NC_CACHE = None


def kernel(x, conv_w, conv_b, bn_gamma, bn_beta, bn_mean, bn_var, fc_w, fc_b):
    global _NC_CACHE
    x = np.asarray(x, np.float32).reshape(BFULL, H * W)
    wt, sfc, fcb, eye16 = _host_prep(
        np.asarray(conv_w, np.float32), np.asarray(conv_b, np.float32),
        np.asarray(bn_gamma, np.float32), np.asarray(bn_beta, np.float32),
        np.asarray(bn_mean, np.float32), np.asarray(bn_var, np.float32),
        np.asarray(fc_w, np.float32), np.asarray(fc_b, np.float32))
    ims = _host_im2col(x)

    if _NC_CACHE is None:
        _NC_CACHE = _build()
    nc = _NC_CACHE

    in_maps = [
        dict(wt=wt, sfc=sfc, fcb=fcb, id16=eye16,
             **{f"im{bt}": ims[i][bt] for bt in range(BT)})
        for i in range(NCORES)
    ]
    res = run_bass_kernel_spmd(nc, in_maps, core_ids=list(range(NCORES)))
    out = np.concatenate([res.results[i]["out"] for i in range(NCORES)], axis=0)
    return out.astype(np.float32)


# revision 3
# speedup vs baseline: 1.0769x; 1.0265x over previous
"""Trainium2 Bass kernel v3 for the binarized CNN:
conv3x3(sign weights) -> BN -> ternary hardtanh -> maxpool4 -> linear(sign weights)

v3 scheduling changes over the v1 baseline (same numerics / host prep):
  - Startup: no PE warmup; wt + im0 chunks ride the sync hardware-DGE
    queue (gpsimd's software DGE is ~40GB/s - too slow for the critical
    path), im1 rides the scalar hardware queue; first conv matmul ~9.5us
    instead of ~17us.
  - The two w-pool levels are batched across each h3's two half-pairs as
    single 3D-strided DVE ops (one [128,2,576] max + one [128,2,288] max
    instead of four 2D ops) - saves ~1us of DVE time per batch tile.
  - FC is interleaved per batch tile: the FC accumulator and the output
    transpose tile rotate through the same PSUM tag (same bank) as the
    transpose tile po, so no separate FC phase / pool-swap drain barrier.
  - Bias via activation into an fp16 row tile, 10x10-identity PE
    transposes (id32 dropped), per-bt output DMA - short serial tail.
"""

import numpy as np
from contextlib import ExitStack

import concourse.bass as bass
import concourse.tile as tile
from concourse import bacc, mybir
from concourse.bass_utils import run_bass_kernel_spmd

F32 = mybir.dt.float32
F16 = mybir.dt.float16
ALU = mybir.AluOpType
ACT = mybir.ActivationFunctionType

NCORES = 8
BFULL = 4096
B = BFULL // NCORES          # 512 per core
P = 128
BT = B // P                  # 4 batch tiles
H, W = 14, 38
HO, WO = 12, 36
C = 32
KP = 116                     # 114 patch rows + 2 threshold-ones rows
NF = C * WO                  # 1152 conv outputs per (b, h)
FP = 864                     # pooled features per sample (3 h3 x 288)
EPS = 1e-5
NOUT = 10
FCJ = (128, 128, 128, 128, 128, 128, 96)   # 864 split into 7 K-chunks

# ---- schedule config ----
# pooling path per pair: 'S' = scalar converts both halves + tail, vector
# maxes fp16; 'H' = scalar converts h0 only, vector maxes h1 from PSUM.
PAIR_ENG = [
    ['S', 'H', 'H', 'H', 'S', 'H'],
    ['S', 'H', 'S', 'H', 'S', 'H'],
    ['S', 'H', 'H', 'H', 'S', 'H'],
    ['S', 'H', 'H', 'H', 'S', 'H'],
]
COPY_ENG = ['S', 'S', 'S', 'S']   # po->tTall copy engine per bt


def _host_prep(conv_w, conv_b, bn_gamma, bn_beta, bn_mean, bn_var, fc_w, fc_b):
    inv = (bn_gamma.astype(np.float64) / np.sqrt(bn_var.astype(np.float64) + EPS))
    thi = (0.5 - bn_beta) / inv + bn_mean - conv_b       # [32] float64
    tlo = (-0.5 - bn_beta) / inv + bn_mean - conv_b
    # fold 1/(thi-tlo) into the weights so the pooled ternary thresholds
    # become the constants 0 (hi, exact: ones-rows carry the split shift)
    # and -1 (lo, ~2^-12 off which is safe in the far tail)
    q16 = np.float16(1.0 / (thi - tlo))
    u64 = thi * q16.astype(np.float64)
    u_h = np.float16(u64)
    u_l = np.float16(u64 - u_h.astype(np.float64))
    sw = np.sign(conv_w[:, 0]).astype(np.float32)        # [32,3,3]

    # phase-major columns: col = (w%4)*288 + c*9 + (w//4) so the w-pool is
    # a max over 4 contiguous 288 blocks (fast DVE tensor_tensor)
    wt = np.zeros((KP, NF), np.float16)
    for c in range(C):
        for w in range(WO):
            n = (w % 4) * 288 + c * 9 + (w // 4)
            for i in range(3):
                for j in range(3):
                    wt[i * W + w + j, n] = np.float16(sw[c, i, j] * q16[c])
            wt[114, n] = -u_h[c]
            wt[115, n] = -u_l[c]

    # pooled feature order (ours): f = h3*288 + c*9 + g ; reference flatten:
    # f_ref = c*27 + h3*9 + g
    sf = np.sign(fc_w).astype(np.float16)                # [10, 864]
    sfc = np.zeros((P, 7 * NOUT), np.float16)
    for j, kj in enumerate(FCJ):
        for r in range(kj):
            f = j * 128 + r
            h3, rem = f // 288, f % 288
            c, g = rem // 9, rem % 9
            fref = c * 27 + h3 * 9 + g
            sfc[r, j * NOUT:(j + 1) * NOUT] = sf[:, fref]

    fcb = fc_b.astype(np.float32).reshape(NOUT, 1)
    eye16 = np.eye(P, dtype=np.float16)
    return wt, sfc, fcb, eye16


def _host_im2col(x):
    """x [4096, 532] fp32 -> per (core, bt) im tiles [116, 1536] fp16."""
    xh = np.float16(x)                                   # [4096, 532]
    win = np.lib.stride_tricks.as_strided(
        xh, shape=(BFULL, HO, 114),
        strides=(xh.strides[0], W * 2, 2))
    ims = []
    for core in range(NCORES):
        row = []
        for bt in range(BT):
            s = core * B + bt * P
            blk = np.transpose(win[s:s + P], (2, 1, 0))  # [114, 12, 128]
            im = np.empty((KP, HO * P), np.float16)
            im[0:114] = blk.reshape(114, HO * P)
            im[114:116] = 1.0
            row.append(np.ascontiguousarray(im))
        ims.append(row)
    return ims


def _build():
    nc = bacc.Bacc("TRN2", target_bir_lowering=False, debug=False,
                   num_devices=NCORES)
    im_d = [nc.dram_tensor(f"im{bt}", [KP, HO * P], F16,
                           kind="ExternalInput").ap() for bt in range(BT)]
    wt_d = nc.dram_tensor("wt", [KP, NF], F16, kind="ExternalInput").ap()
    wt0s_d = nc.dram_tensor("wt0s", [KP, 512], F16, kind="ExternalInput").ap()
    sfc_d = nc.dram_tensor("sfc", [P, 7 * NOUT], F16, kind="ExternalInput").ap()
    fcb_d = nc.dram_tensor("fcb", [NOUT, 1], F32, kind="ExternalInput").ap()
    id16_d = nc.dram_tensor("id16", [P, P], F16, kind="ExternalInput").ap()
    out_d = nc.dram_tensor("out", [B, NOUT], F32, kind="ExternalOutput").ap()

    with tile.TileContext(nc) as tc, ExitStack() as ctx:
        const = ctx.enter_context(tc.tile_pool(name="const", bufs=1))
        imp = ctx.enter_context(tc.tile_pool(name="imp", bufs=1))
        zsp = ctx.enter_context(tc.tile_pool(name="zsp", bufs=4))
        msp = ctx.enter_context(tc.tile_pool(name="msp", bufs=6))
        upp = ctx.enter_context(tc.tile_pool(name="upp", bufs=2))
        mtp = ctx.enter_context(tc.tile_pool(name="mtp", bufs=2))
        ttp = ctx.enter_context(tc.tile_pool(name="ttp", bufs=1))

        wt = const.tile([KP, NF], F16, tag="wt")
        wt0s = const.tile([KP, 512], F16, tag="wt0s")
        sfc = const.tile([P, 7 * NOUT], F16, tag="sfc")
        fcb = const.tile([NOUT, 1], F32, tag="fcb")
        id16 = const.tile([P, P], F16, tag="id16")
        ob = const.tile([16, B], F16, tag="ob")
        ims = [imp.tile([KP, HO * P], F16, tag=f"im{bt}", name=f"im{bt}")
               for bt in range(BT)]

        # ---- DMA issue schedule. Measured queue behavior: descriptor
        # processing is ~92ns/row regardless of row size, so full-tile
        # transfers (2-3KB rows) are 2-3x faster per byte than column
        # chunks; the sync hw queue starts ~8.8us, the scalar hw queue
        # ~10.9us but with priority over sync; gpsimd's software queue
        # spreads wide across engines. wt rides the scalar queue as ONE
        # full-tile transfer, and a small duplicate of its first 512
        # columns rides gpsimd so the A-chunk matmuls can start early.
        nc.sync.dma_start(ims[0][:, 0:512], im_d[0][:, 0:512])
        nc.sync.dma_start(ims[0][:, 512:1024], im_d[0][:, 512:1024])
        nc.sync.dma_start(ims[3][:], im_d[3])
        nc.scalar.dma_start(wt[:], wt_d)
        nc.scalar.dma_start(ims[0][:, 1024:1536], im_d[0][:, 1024:1536])
        nc.scalar.dma_start(ims[1][:], im_d[1])
        nc.scalar.dma_start(ims[2][:], im_d[2])
        nc.gpsimd.dma_start(wt0s[:], wt0s_d)
        nc.gpsimd.dma_start(sfc[:], sfc_d)
        nc.gpsimd.dma_start(fcb[:], fcb_d)
        nc.gpsimd.dma_start(id16[:], id16_d)

        # bt-major transposed ternary: bt block = [j(7) x 128 batch cols]
        tTall = ttp.tile([P, BT * 7 * P], F16, tag="tTall")

        with tc.tile_pool(name="zp", bufs=3, space="PSUM") as zp, \
             tc.tile_pool(name="ztp", bufs=1, space="PSUM") as ztp, \
             tc.tile_pool(name="pop", bufs=1, space="PSUM") as pop:

            def emit_fc_acc(bt):
                """FC matmuls + bias-activation for bt (acc rotates through
                the po PSUM tag/bank)."""
                accb = pop.tile([NOUT, P], F32, tag="po", name=f"acc{bt}")
                for j, kj in enumerate(FCJ):
                    nc.tensor.matmul(
                        accb[:, :],
                        lhsT=sfc[0:kj, j * NOUT:(j + 1) * NOUT],
                        rhs=tTall[0:kj, bt * 7 * P + j * P:
                                  bt * 7 * P + (j + 1) * P],
                        start=(j == 0), stop=(j == 6))
                nc.scalar.activation(ob[0:NOUT, bt * P:(bt + 1) * P],
                                     accb[:, :], ACT.Identity,
                                     bias=fcb[0:NOUT, 0:1], scale=1.0)

            def emit_fc_out(bt):
                """Transpose the fp16 FC row back to batch-major and DMA."""
                po2 = pop.tile([P, NOUT], F16, tag="po", name=f"po2{bt}")
                nc.tensor.transpose(po2[0:P, 0:NOUT],
                                    ob[0:NOUT, bt * P:(bt + 1) * P],
                                    id16[0:NOUT, 0:NOUT])
                os_ = const.tile([P, NOUT], F32, tag=f"os{bt}", name=f"os{bt}")
                nc.scalar.copy(os_[:], po2[:, 0:NOUT])
                nc.sync.dma_start(out_d[bt * P:(bt + 1) * P, :], os_[:])

            for bt in range(BT):
                # up3[p, half, h3*288+cg] - same layout as a flat half-major
                # [P, 1728] tile, 3D-viewed for the batched w-pool writes
                up3 = upp.tile([P, 2, 3 * 288], F16, tag="up", name=f"up{bt}")
                mt = mtp.tile([P, FP], F16, tag="mt", name=f"mt{bt}")
                lt = mtp.tile([P, FP], F16, tag="lt", name=f"lt{bt}")
                t_ = mtp.tile([P, FP], F16, tag="t_", name=f"t{bt}")
                po = None
                for h3 in range(3):
                    # m2[p, half, 1152]: both half-pairs of this h3 group
                    m2 = msp.tile([P, 2, NF], F16, tag="m2",
                                  name=f"m2{bt}_{h3}")
                    # zt4[p, half, 2*128]: all four tail chunks of the h3
                    # group share one PSUM bank, converted by ONE scalar op
                    zt4 = ztp.tile([P, 2, 256], F32, tag="zt",
                                   name=f"zt{bt}_{h3}")
                    for half in range(2):
                        pi = h3 * 2 + half
                        zh = []
                        for hh in range(2):
                            h = 4 * h3 + 2 * half + hh
                            k = h * P
                            z = zp.tile([P, 1024], F32, tag="z",
                                        name=f"z{bt}_{h}")
                            # A-chunk reads the duplicate wt0s tile (same
                            # bytes as wt[:,0:512]) which lands earliest
                            nc.tensor.matmul(
                                z[:, 0:512],
                                lhsT=ims[bt][:, k:k + P],
                                rhs=wt0s[:, 0:512],
                                start=True, stop=True)
                            nc.tensor.matmul(
                                z[:, 512:1024],
                                lhsT=ims[bt][:, k:k + P],
                                rhs=wt[:, 512:1024],
                                start=True, stop=True)
                            nc.tensor.matmul(
                                zt4[:, half, hh * 128:(hh + 1) * 128],
                                lhsT=ims[bt][:, k:k + P],
                                rhs=wt[:, 1024:NF],
                                start=True, stop=True)
                            zh.append(z)
                        eng = PAIR_ENG[bt][pi]
                        if eng == 'S':
                            zs = zsp.tile([P, 2048], F16, tag="zs",
                                          name=f"zs{bt}_{pi}")
                            nc.scalar.copy(zs[:, 0:1024], zh[0][:, 0:1024])
                            nc.scalar.copy(zs[:, 1024:2048], zh[1][:, 0:1024])
                            nc.vector.tensor_max(m2[:, half, 0:1024],
                                                 zs[:, 0:1024],
                                                 zs[:, 1024:2048])
                        else:  # H: scalar converts h0, vector TTs psum h1
                            m0 = msp.tile([P, 1024], F16, tag="m0",
                                          name=f"m0{bt}_{pi}")
                            nc.scalar.copy(m0[:, 0:1024], zh[0][:, 0:1024])
                            nc.vector.tensor_max(m2[:, half, 0:1024],
                                                 zh[1][:, 0:1024],
                                                 m0[:, 0:1024])
                        # deferred FC of the previous bt: its tTall copy is
                        # ~2 pairs old by now, so the PE queue won't block.
                        # Split so the po2 transpose (which waits on the
                        # scalar act) sits another pair later in PE order.
                        if bt > 0 and h3 == 0 and half == 1:
                            emit_fc_acc(bt - 1)
                        if bt > 0 and h3 == 1 and half == 0:
                            emit_fc_out(bt - 1)
                    # group tail: one scalar convert of all 4 tail chunks,
                    # one 3D-strided vector max into both halves' m2 tails
                    ztf = zsp.tile([P, 2, 256], F16, tag="ztf",
                                   name=f"ztf{bt}_{h3}")
                    nc.scalar.copy(ztf[:, :, :], zt4[:, :, :])
                    nc.vector.tensor_max(m2[:, :, 1024:NF],
                                         ztf[:, :, 0:128], ztf[:, :, 128:256])
                    # batched w-pool for both halves of this h3 group:
                    # level 1: [P,2,576], level 2: [P,2,288] into up slots
                    n4 = msp.tile([P, 2, 576], F16, tag="n4",
                                  name=f"n4{bt}_{h3}")
                    nc.vector.tensor_max(n4[:, :, :], m2[:, :, 0:576],
                                         m2[:, :, 576:NF])
                    nc.vector.tensor_max(up3[:, :, h3 * 288:(h3 + 1) * 288],
                                         n4[:, :, 0:288], n4[:, :, 288:576])

                    # bt3: finalize h3 groups early to shorten the tail
                    if bt == BT - 1 and h3 >= 1:
                        if h3 == 1:
                            c0, c1 = 0, 576
                            jlist = [0, 1, 2, 3]
                        else:
                            c0, c1 = 576, 864
                            jlist = [4, 5, 6]
                        _finalize(nc, up3, mt, lt, t_, c0, c1)
                        if po is None:
                            po = pop.tile([P, 7 * P], F16, tag="po",
                                          name=f"po{bt}")
                        for j in jlist:
                            kj = FCJ[j]
                            nc.tensor.transpose(
                                po[0:kj, j * P:(j + 1) * P],
                                t_[:, j * 128:j * 128 + kj], id16[:])
                        ccol = (jlist[0] * P, (jlist[-1] + 1) * P)
                        nc.scalar.copy(
                            tTall[:, bt * 7 * P + ccol[0]:
                                  bt * 7 * P + ccol[1]],
                            po[:, ccol[0]:ccol[1]])

                if bt < BT - 1:
                    _finalize(nc, up3, mt, lt, t_, 0, FP)
                    po = pop.tile([P, 7 * P], F16, tag="po", name=f"po{bt}")
                    for j, kj in enumerate(FCJ):
                        nc.tensor.transpose(po[0:kj, j * P:(j + 1) * P],
                                            t_[:, j * 128:j * 128 + kj],
                                            id16[:])
                    if COPY_ENG[bt] == 'V':
                        nc.vector.tensor_copy(
                            tTall[:, bt * 7 * P:(bt + 1) * 7 * P], po[:, :])
                    else:
                        nc.scalar.copy(
                            tTall[:, bt * 7 * P:(bt + 1) * 7 * P], po[:, :])

            # bt3's FC runs in the tail
            emit_fc_acc(BT - 1)
            emit_fc_out(BT - 1)

    nc.compile()
    return nc


def _finalize(nc, up3, mt, lt, t_, c0, c1):
    """M = max over halves; lt = M < -1; t = (M > 0) - lt, on cols [c0,c1)."""
    nc.vector.tensor_max(mt[:, c0:c1], up3[:, 0, c0:c1], up3[:, 1, c0:c1])
    nc.vector.tensor_scalar(lt[:, c0:c1], mt[:, c0:c1], -1.0, None,
                            ALU.is_lt)
    nc.vector.scalar_tensor_tensor(
        t_[:, c0:c1], mt[:, c0:c1], 0.0, lt[:, c0:c1],
        ALU.is_gt, ALU.subtract)


_NC_CACHE = None


def kernel(x, conv_w, conv_b, bn_gamma, bn_beta, bn_mean, bn_var, fc_w, fc_b):
    global _NC_CACHE
    x = np.asarray(x, np.float32).reshape(BFULL, H * W)
    wt, sfc, fcb, eye16 = _host_prep(
        np.asarray(conv_w, np.float32), np.asarray(conv_b, np.float32),
        np.asarray(bn_gamma, np.float32), np.asarray(bn_beta, np.float32),
        np.asarray(bn_mean, np.float32), np.asarray(bn_var, np.float32),
        np.asarray(fc_w, np.float32), np.asarray(fc_b, np.float32))
    ims = _host_im2col(x)

    if _NC_CACHE is None:
        _NC_CACHE = _build()
    nc = _NC_CACHE

    wt0s = np.ascontiguousarray(wt[:, 0:512])
    in_maps = [
        dict(wt=wt, wt0s=wt0s, sfc=sfc, fcb=fcb, id16=eye16,
             **{f"im{bt}": ims[i][bt] for bt in range(BT)})
        for i in range(NCORES)
    ]
    res = run_bass_kernel_spmd(nc, in_maps, core_ids=list(range(NCORES)))
    out = np.concatenate([res.results[i]["out"] for i in range(NCORES)], axis=0)
    return out.astype(np.float32)


# revision 4
# speedup vs baseline: 1.0891x; 1.0113x over previous
"""Trainium2 Bass kernel v3 for the binarized CNN:
conv3x3(sign weights) -> BN -> ternary hardtanh -> maxpool4 -> linear(sign weights)

v3 scheduling changes over the v1 baseline (same numerics / host prep):
  - Startup: no PE warmup; wt + im0 chunks ride the sync hardware-DGE
    queue (gpsimd's software DGE is ~40GB/s - too slow for the critical
    path), im1 rides the scalar hardware queue; first conv matmul ~9.5us
    instead of ~17us.
  - The two w-pool levels are batched across each h3's two half-pairs as
    single 3D-strided DVE ops (one [128,2,576] max + one [128,2,288] max
    instead of four 2D ops) - saves ~1us of DVE time per batch tile.
  - FC is interleaved per batch tile: the FC accumulator and the output
    transpose tile rotate through the same PSUM tag (same bank) as the
    transpose tile po, so no separate FC phase / pool-swap drain barrier.
  - Bias via activation into an fp16 row tile, 10x10-identity PE
    transposes (id32 dropped), per-bt output DMA - short serial tail.
"""

import numpy as np
from contextlib import ExitStack

import concourse.bass as bass
import concourse.tile as tile
from concourse import bacc, mybir
from concourse.bass_utils import run_bass_kernel_spmd

F32 = mybir.dt.float32
F16 = mybir.dt.float16
ALU = mybir.AluOpType
ACT = mybir.ActivationFunctionType

NCORES = 8
BFULL = 4096
B = BFULL // NCORES          # 512 per core
P = 128
BT = B // P                  # 4 batch tiles
H, W = 14, 38
HO, WO = 12, 36
C = 32
KP = 116                     # 114 patch rows + 2 threshold-ones rows
NF = C * WO                  # 1152 conv outputs per (b, h)
FP = 864                     # pooled features per sample (3 h3 x 288)
EPS = 1e-5
NOUT = 10
FCJ = (128, 128, 128, 128, 128, 128, 96)   # 864 split into 7 K-chunks

# ---- schedule config ----
# pooling path per pair: 'S' = scalar converts both halves + tail, vector
# maxes fp16; 'H' = scalar converts h0 only, vector maxes h1 from PSUM.
PAIR_ENG = [
    ['S', 'H', 'H', 'H', 'S', 'H'],
    ['S', 'H', 'S', 'H', 'S', 'H'],
    ['S', 'H', 'H', 'H', 'S', 'H'],
    ['S', 'H', 'H', 'H', 'S', 'H'],
]
COPY_ENG = ['S', 'S', 'S', 'S']   # po->tTall copy engine per bt


def _host_prep(conv_w, conv_b, bn_gamma, bn_beta, bn_mean, bn_var, fc_w, fc_b):
    inv = (bn_gamma.astype(np.float64) / np.sqrt(bn_var.astype(np.float64) + EPS))
    thi = (0.5 - bn_beta) / inv + bn_mean - conv_b       # [32] float64
    tlo = (-0.5 - bn_beta) / inv + bn_mean - conv_b
    # fold 1/(thi-tlo) into the weights so the pooled ternary thresholds
    # become the constants 0 (hi, exact: ones-rows carry the split shift)
    # and -1 (lo, ~2^-12 off which is safe in the far tail)
    q16 = np.float16(1.0 / (thi - tlo))
    u64 = thi * q16.astype(np.float64)
    u_h = np.float16(u64)
    u_l = np.float16(u64 - u_h.astype(np.float64))
    sw = np.sign(conv_w[:, 0]).astype(np.float32)        # [32,3,3]

    # phase-major columns: col = (w%4)*288 + c*9 + (w//4) so the w-pool is
    # a max over 4 contiguous 288 blocks (fast DVE tensor_tensor)
    wt = np.zeros((KP, NF), np.float16)
    for c in range(C):
        for w in range(WO):
            n = (w % 4) * 288 + c * 9 + (w // 4)
            for i in range(3):
                for j in range(3):
                    wt[i * W + w + j, n] = np.float16(sw[c, i, j] * q16[c])
            wt[114, n] = -u_h[c]
            wt[115, n] = -u_l[c]

    # pooled feature order (ours): f = h3*288 + c*9 + g ; reference flatten:
    # f_ref = c*27 + h3*9 + g
    sf = np.sign(fc_w).astype(np.float16)                # [10, 864]
    sfc = np.zeros((P, 7 * NOUT), np.float16)
    for j, kj in enumerate(FCJ):
        for r in range(kj):
            f = j * 128 + r
            h3, rem = f // 288, f % 288
            c, g = rem // 9, rem % 9
            fref = c * 27 + h3 * 9 + g
            sfc[r, j * NOUT:(j + 1) * NOUT] = sf[:, fref]

    fcb = fc_b.astype(np.float32).reshape(NOUT, 1)
    eye16 = np.eye(P, dtype=np.float16)
    return wt, sfc, fcb, eye16


def _host_im2col(x):
    """x [4096, 532] fp32 -> per (core, bt) im tiles [116, 1536] fp16."""
    xh = np.float16(x)                                   # [4096, 532]
    win = np.lib.stride_tricks.as_strided(
        xh, shape=(BFULL, HO, 114),
        strides=(xh.strides[0], W * 2, 2))
    ims = []
    for core in range(NCORES):
        row = []
        for bt in range(BT):
            s = core * B + bt * P
            blk = np.transpose(win[s:s + P], (2, 1, 0))  # [114, 12, 128]
            im = np.empty((KP, HO * P), np.float16)
            im[0:114] = blk.reshape(114, HO * P)
            im[114:116] = 1.0
            row.append(np.ascontiguousarray(im))
        ims.append(row)
    return ims


def _build():
    nc = bacc.Bacc("TRN2", target_bir_lowering=False, debug=False,
                   num_devices=NCORES)
    im_d = [nc.dram_tensor(f"im{bt}", [KP, HO * P], F16,
                           kind="ExternalInput").ap() for bt in range(BT)]
    wt_d = nc.dram_tensor("wt", [KP, NF], F16, kind="ExternalInput").ap()
    sfc_d = nc.dram_tensor("sfc", [P, 7 * NOUT], F16, kind="ExternalInput").ap()
    fcb_d = nc.dram_tensor("fcb", [NOUT, 1], F32, kind="ExternalInput").ap()
    id16_d = nc.dram_tensor("id16", [P, P], F16, kind="ExternalInput").ap()
    out_d = nc.dram_tensor("out", [B, NOUT], F32, kind="ExternalOutput").ap()

    with tile.TileContext(nc) as tc, ExitStack() as ctx:
        const = ctx.enter_context(tc.tile_pool(name="const", bufs=1))
        imp = ctx.enter_context(tc.tile_pool(name="imp", bufs=1))
        zsp = ctx.enter_context(tc.tile_pool(name="zsp", bufs=4))
        msp = ctx.enter_context(tc.tile_pool(name="msp", bufs=6))
        upp = ctx.enter_context(tc.tile_pool(name="upp", bufs=2))
        mtp = ctx.enter_context(tc.tile_pool(name="mtp", bufs=2))
        ttp = ctx.enter_context(tc.tile_pool(name="ttp", bufs=1))

        wt = const.tile([KP, NF], F16, tag="wt")
        sfc = const.tile([P, 7 * NOUT], F16, tag="sfc")
        fcb = const.tile([NOUT, 1], F32, tag="fcb")
        id16 = const.tile([P, P], F16, tag="id16")
        ob = const.tile([16, B], F16, tag="ob")
        ims = [imp.tile([KP, HO * P], F16, tag=f"im{bt}", name=f"im{bt}")
               for bt in range(BT)]

        # ---- DMA issue schedule. Measured queue behavior: descriptor
        # processing is ~92ns/row regardless of row size, so full-tile
        # transfers (2-3KB rows) are 2-3x faster per byte than column
        # chunks; the sync hw queue starts ~8.8us, the scalar hw queue
        # ~10.9us but with priority over sync; gpsimd's software queue
        # spreads wide across engines. wt rides the scalar queue as ONE
        # full-tile transfer, and a small duplicate of its first 512
        # columns rides gpsimd so the A-chunk matmuls can start early.
        nc.sync.dma_start(ims[0][:, 0:512], im_d[0][:, 0:512])
        nc.sync.dma_start(ims[0][:, 512:1024], im_d[0][:, 512:1024])
        nc.sync.dma_start(ims[3][:], im_d[3])
        nc.scalar.dma_start(wt[:], wt_d)
        nc.scalar.dma_start(ims[0][:, 1024:1536], im_d[0][:, 1024:1536])
        nc.scalar.dma_start(ims[1][:], im_d[1])
        nc.scalar.dma_start(ims[2][:], im_d[2])
        nc.gpsimd.dma_start(sfc[:], sfc_d)
        nc.gpsimd.dma_start(fcb[:], fcb_d)
        nc.gpsimd.dma_start(id16[:], id16_d)

        # bt-major transposed ternary: bt block = [j(7) x 128 batch cols]
        tTall = ttp.tile([P, BT * 7 * P], F16, tag="tTall")

        with tc.tile_pool(name="zp", bufs=3, space="PSUM") as zp, \
             tc.tile_pool(name="ztp", bufs=1, space="PSUM") as ztp, \
             tc.tile_pool(name="pop", bufs=1, space="PSUM") as pop:

            def emit_fc_acc(bt):
                """FC matmuls + bias-activation for bt (acc rotates through
                the po PSUM tag/bank)."""
                accb = pop.tile([NOUT, P], F32, tag="po", name=f"acc{bt}")
                for j, kj in enumerate(FCJ):
                    nc.tensor.matmul(
                        accb[:, :],
                        lhsT=sfc[0:kj, j * NOUT:(j + 1) * NOUT],
                        rhs=tTall[0:kj, bt * 7 * P + j * P:
                                  bt * 7 * P + (j + 1) * P],
                        start=(j == 0), stop=(j == 6))
                nc.scalar.activation(ob[0:NOUT, bt * P:(bt + 1) * P],
                                     accb[:, :], ACT.Identity,
                                     bias=fcb[0:NOUT, 0:1], scale=1.0)

            def emit_fc_out(bt):
                """Transpose the fp16 FC row back to batch-major and DMA."""
                po2 = pop.tile([P, NOUT], F16, tag="po", name=f"po2{bt}")
                nc.tensor.transpose(po2[0:P, 0:NOUT],
                                    ob[0:NOUT, bt * P:(bt + 1) * P],
                                    id16[0:NOUT, 0:NOUT])
                os_ = const.tile([P, NOUT], F32, tag=f"os{bt}", name=f"os{bt}")
                nc.scalar.copy(os_[:], po2[:, 0:NOUT])
                nc.sync.dma_start(out_d[bt * P:(bt + 1) * P, :], os_[:])

            def emit_finalize_block(fbt, fup3):
                mt = mtp.tile([P, FP], F16, tag="mt", name=f"mt{fbt}")
                lt = mtp.tile([P, FP], F16, tag="lt", name=f"lt{fbt}")
                t_ = mtp.tile([P, FP], F16, tag="t_", name=f"t{fbt}")
                _finalize(nc, fup3, mt, lt, t_, 0, FP)
                return t_

            def emit_transposes_copy(fbt, t_):
                po = pop.tile([P, 7 * P], F16, tag="po", name=f"po{fbt}")
                for j, kj in enumerate(FCJ):
                    nc.tensor.transpose(po[0:kj, j * P:(j + 1) * P],
                                        t_[:, j * 128:j * 128 + kj],
                                        id16[:])
                nc.scalar.copy(
                    tTall[:, fbt * 7 * P:(fbt + 1) * 7 * P], po[:, :])

            pend = None
            for bt in range(BT):
                # up3[p, half, h3*288+cg] - same layout as a flat half-major
                # [P, 1728] tile, 3D-viewed for the batched w-pool writes
                up3 = upp.tile([P, 2, 3 * 288], F16, tag="up", name=f"up{bt}")
                mt = lt = t_ = None
                po = None
                for h3 in range(3):
                    # m2[p, half, 1152]: both half-pairs of this h3 group
                    m2 = msp.tile([P, 2, NF], F16, tag="m2",
                                  name=f"m2{bt}_{h3}")
                    # zt4[p, half, 2*128]: all four tail chunks of the h3
                    # group share one PSUM bank, converted by ONE scalar op
                    zt4 = ztp.tile([P, 2, 256], F32, tag="zt",
                                   name=f"zt{bt}_{h3}")
                    for half in range(2):
                        pi = h3 * 2 + half
                        zh = []
                        for hh in range(2):
                            h = 4 * h3 + 2 * half + hh
                            k = h * P
                            z = zp.tile([P, 1024], F32, tag="z",
                                        name=f"z{bt}_{h}")
                            for n0, n1 in ((0, 512), (512, 1024)):
                                nc.tensor.matmul(
                                    z[:, n0:n1],
                                    lhsT=ims[bt][:, k:k + P],
                                    rhs=wt[:, n0:n1],
                                    start=True, stop=True)
                            nc.tensor.matmul(
                                zt4[:, half, hh * 128:(hh + 1) * 128],
                                lhsT=ims[bt][:, k:k + P],
                                rhs=wt[:, 1024:NF],
                                start=True, stop=True)
                            zh.append(z)
                        eng = PAIR_ENG[bt][pi]
                        if eng == 'S':
                            zs = zsp.tile([P, 2048], F16, tag="zs",
                                          name=f"zs{bt}_{pi}")
                            nc.scalar.copy(zs[:, 0:1024], zh[0][:, 0:1024])
                            nc.scalar.copy(zs[:, 1024:2048], zh[1][:, 0:1024])
                            nc.vector.tensor_max(m2[:, half, 0:1024],
                                                 zs[:, 0:1024],
                                                 zs[:, 1024:2048])
                        else:  # H: scalar converts h0, vector TTs psum h1
                            m0 = msp.tile([P, 1024], F16, tag="m0",
                                          name=f"m0{bt}_{pi}")
                            nc.scalar.copy(m0[:, 0:1024], zh[0][:, 0:1024])
                            nc.vector.tensor_max(m2[:, half, 0:1024],
                                                 zh[1][:, 0:1024],
                                                 m0[:, 0:1024])
                        # deferred pipeline of the previous bt, spread over
                        # this bt's first four pairs so the vector/PE queues
                        # never see a burst at the bt boundary:
                        #   (0,0) finalize  (0,1) transposes+copy
                        #   (1,0) FC acc    (1,1) FC out
                        if pend is not None:
                            if h3 == 0 and half == 0:
                                pend['t_'] = emit_finalize_block(
                                    pend['bt'], pend['up3'])
                            elif h3 == 0 and half == 1:
                                emit_transposes_copy(pend['bt'], pend['t_'])
                            elif h3 == 1 and half == 0:
                                emit_fc_acc(pend['bt'])
                            elif h3 == 1 and half == 1:
                                emit_fc_out(pend['bt'])
                                pend = None
                    # group tail: one scalar convert of all 4 tail chunks,
                    # one 3D-strided vector max into both halves' m2 tails
                    ztf = zsp.tile([P, 2, 256], F16, tag="ztf",
                                   name=f"ztf{bt}_{h3}")
                    nc.scalar.copy(ztf[:, :, :], zt4[:, :, :])
                    nc.vector.tensor_max(m2[:, :, 1024:NF],
                                         ztf[:, :, 0:128], ztf[:, :, 128:256])
                    # batched w-pool for both halves of this h3 group:
                    # level 1: [P,2,576], level 2: [P,2,288] into up slots
                    n4 = msp.tile([P, 2, 576], F16, tag="n4",
                                  name=f"n4{bt}_{h3}")
                    nc.vector.tensor_max(n4[:, :, :], m2[:, :, 0:576],
                                         m2[:, :, 576:NF])
                    nc.vector.tensor_max(up3[:, :, h3 * 288:(h3 + 1) * 288],
                                         n4[:, :, 0:288], n4[:, :, 288:576])

                    # bt3: finalize h3 groups early to shorten the tail
                    if bt == BT - 1 and h3 >= 1:
                        if h3 == 1:
                            c0, c1 = 0, 576
                            jlist = [0, 1, 2, 3]
                            mt = mtp.tile([P, FP], F16, tag="mt",
                                          name=f"mt{bt}")
                            lt = mtp.tile([P, FP], F16, tag="lt",
                                          name=f"lt{bt}")
                            t_ = mtp.tile([P, FP], F16, tag="t_",
                                          name=f"t{bt}")
                        else:
                            c0, c1 = 576, 864
                            jlist = [4, 5, 6]
                        _finalize(nc, up3, mt, lt, t_, c0, c1)
                        if po is None:
                            po = pop.tile([P, 7 * P], F16, tag="po",
                                          name=f"po{bt}")
                        for j in jlist:
                            kj = FCJ[j]
                            nc.tensor.transpose(
                                po[0:kj, j * P:(j + 1) * P],
                                t_[:, j * 128:j * 128 + kj], id16[:])
                        ccol = (jlist[0] * P, (jlist[-1] + 1) * P)
                        nc.scalar.copy(
                            tTall[:, bt * 7 * P + ccol[0]:
                                  bt * 7 * P + ccol[1]],
                            po[:, ccol[0]:ccol[1]])

                if bt < BT - 1:
                    pend = {'bt': bt, 'up3': up3}

            # bt3's FC runs in the tail
            emit_fc_acc(BT - 1)
            emit_fc_out(BT - 1)

    nc.compile()
    return nc


def _finalize(nc, up3, mt, lt, t_, c0, c1):
    """M = max over halves; lt = M < -1; t = (M > 0) - lt, on cols [c0,c1)."""
    nc.vector.tensor_max(mt[:, c0:c1], up3[:, 0, c0:c1], up3[:, 1, c0:c1])
    nc.vector.tensor_scalar(lt[:, c0:c1], mt[:, c0:c1], -1.0, None,
                            ALU.is_lt)
    nc.vector.scalar_tensor_tensor(
        t_[:, c0:c1], mt[:, c0:c1], 0.0, lt[:, c0:c1],
        ALU.is_gt, ALU.subtract)


_NC_CACHE = None


def kernel(x, conv_w, conv_b, bn_gamma, bn_beta, bn_mean, bn_var, fc_w, fc_b):
    global _NC_CACHE
    x = np.asarray(x, np.float32).reshape(BFULL, H * W)
    wt, sfc, fcb, eye16 = _host_prep(
        np.asarray(conv_w, np.float32), np.asarray(conv_b, np.float32),
        np.asarray(bn_gamma, np.float32), np.asarray(bn_beta, np.float32),
        np.asarray(bn_mean, np.float32), np.asarray(bn_var, np.float32),
        np.asarray(fc_w, np.float32), np.asarray(fc_b, np.float32))
    ims = _host_im2col(x)

    if _NC_CACHE is None:
        _NC_CACHE = _build()
    nc = _NC_CACHE

    in_maps = [
        dict(wt=wt, sfc=sfc, fcb=fcb, id16=eye16,
             **{f"im{bt}": ims[i][bt] for bt in range(BT)})
        for i in range(NCORES)
    ]
    res = run_bass_kernel_spmd(nc, in_maps, core_ids=list(range(NCORES)))
    out = np.concatenate([res.results[i]["out"] for i in range(NCORES)], axis=0)
    return out.astype(np.float32)


# revision 5
# speedup vs baseline: 1.0921x; 1.0028x over previous
"""Trainium2 Bass kernel v3 for the binarized CNN:
conv3x3(sign weights) -> BN -> ternary hardtanh -> maxpool4 -> linear(sign weights)

v3 scheduling changes over the v1 baseline (same numerics / host prep):
  - Startup: no PE warmup; wt + im0 chunks ride the sync hardware-DGE
    queue (gpsimd's software DGE is ~40GB/s - too slow for the critical
    path), im1 rides the scalar hardware queue; first conv matmul ~9.5us
    instead of ~17us.
  - The two w-pool levels are batched across each h3's two half-pairs as
    single 3D-strided DVE ops (one [128,2,576] max + one [128,2,288] max
    instead of four 2D ops) - saves ~1us of DVE time per batch tile.
  - FC is interleaved per batch tile: the FC accumulator and the output
    transpose tile rotate through the same PSUM tag (same bank) as the
    transpose tile po, so no separate FC phase / pool-swap drain barrier.
  - Bias via activation into an fp16 row tile, 10x10-identity PE
    transposes (id32 dropped), per-bt output DMA - short serial tail.
"""

import numpy as np
from contextlib import ExitStack

import concourse.bass as bass
import concourse.tile as tile
from concourse import bacc, mybir
from concourse.bass_utils import run_bass_kernel_spmd

F32 = mybir.dt.float32
F16 = mybir.dt.float16
ALU = mybir.AluOpType
ACT = mybir.ActivationFunctionType

NCORES = 8
BFULL = 4096
B = BFULL // NCORES          # 512 per core
P = 128
BT = B // P                  # 4 batch tiles
H, W = 14, 38
HO, WO = 12, 36
C = 32
KP = 116                     # 114 patch rows + 2 threshold-ones rows
NF = C * WO                  # 1152 conv outputs per (b, h)
FP = 864                     # pooled features per sample (3 h3 x 288)
EPS = 1e-5
NOUT = 10
FCJ = (128, 128, 128, 128, 128, 128, 96)   # 864 split into 7 K-chunks

# ---- schedule config ----
# pooling path per pair: 'S' = scalar converts both halves + tail, vector
# maxes fp16; 'H' = scalar converts h0 only, vector maxes h1 from PSUM.
PAIR_ENG = [
    ['S', 'H', 'H', 'H', 'S', 'H'],
    ['S', 'H', 'S', 'H', 'S', 'H'],
    ['S', 'H', 'H', 'H', 'S', 'H'],
    ['S', 'H', 'H', 'H', 'S', 'H'],
]
COPY_ENG = ['S', 'S', 'S', 'S']   # po->tTall copy engine per bt


def _host_prep(conv_w, conv_b, bn_gamma, bn_beta, bn_mean, bn_var, fc_w, fc_b):
    inv = (bn_gamma.astype(np.float64) / np.sqrt(bn_var.astype(np.float64) + EPS))
    thi = (0.5 - bn_beta) / inv + bn_mean - conv_b       # [32] float64
    tlo = (-0.5 - bn_beta) / inv + bn_mean - conv_b
    # fold 1/(thi-tlo) into the weights so the pooled ternary thresholds
    # become the constants 0 (hi, exact: ones-rows carry the split shift)
    # and -1 (lo, ~2^-12 off which is safe in the far tail)
    q16 = np.float16(1.0 / (thi - tlo))
    u64 = thi * q16.astype(np.float64)
    u_h = np.float16(u64)
    u_l = np.float16(u64 - u_h.astype(np.float64))
    sw = np.sign(conv_w[:, 0]).astype(np.float32)        # [32,3,3]

    # phase-major columns: col = (w%4)*288 + c*9 + (w//4) so the w-pool is
    # a max over 4 contiguous 288 blocks (fast DVE tensor_tensor)
    wt = np.zeros((KP, NF), np.float16)
    for c in range(C):
        for w in range(WO):
            n = (w % 4) * 288 + c * 9 + (w // 4)
            for i in range(3):
                for j in range(3):
                    wt[i * W + w + j, n] = np.float16(sw[c, i, j] * q16[c])
            wt[114, n] = -u_h[c]
            wt[115, n] = -u_l[c]

    # pooled feature order (ours): f = h3*288 + c*9 + g ; reference flatten:
    # f_ref = c*27 + h3*9 + g
    sf = np.sign(fc_w).astype(np.float16)                # [10, 864]
    sfc = np.zeros((P, 7 * NOUT), np.float16)
    for j, kj in enumerate(FCJ):
        for r in range(kj):
            f = j * 128 + r
            h3, rem = f // 288, f % 288
            c, g = rem // 9, rem % 9
            fref = c * 27 + h3 * 9 + g
            sfc[r, j * NOUT:(j + 1) * NOUT] = sf[:, fref]

    fcb = fc_b.astype(np.float32).reshape(NOUT, 1)
    eye16 = np.eye(P, dtype=np.float16)
    return wt, sfc, fcb, eye16


def _host_im2col(x):
    """x [4096, 532] fp32 -> per (core, bt) im tiles [116, 1536] fp16."""
    xh = np.float16(x)                                   # [4096, 532]
    win = np.lib.stride_tricks.as_strided(
        xh, shape=(BFULL, HO, 114),
        strides=(xh.strides[0], W * 2, 2))
    ims = []
    for core in range(NCORES):
        row = []
        for bt in range(BT):
            s = core * B + bt * P
            blk = np.transpose(win[s:s + P], (2, 1, 0))  # [114, 12, 128]
            im = np.empty((KP, HO * P), np.float16)
            im[0:114] = blk.reshape(114, HO * P)
            im[114:116] = 1.0
            row.append(np.ascontiguousarray(im))
        ims.append(row)
    return ims


def _build():
    nc = bacc.Bacc("TRN2", target_bir_lowering=False, debug=False,
                   num_devices=NCORES)
    im_d = [nc.dram_tensor(f"im{bt}", [KP, HO * P], F16,
                           kind="ExternalInput").ap() for bt in range(BT)]
    wt_d = nc.dram_tensor("wt", [KP, NF], F16, kind="ExternalInput").ap()
    sfc_d = nc.dram_tensor("sfc", [P, 7 * NOUT], F16, kind="ExternalInput").ap()
    fcb_d = nc.dram_tensor("fcb", [NOUT, 1], F32, kind="ExternalInput").ap()
    id16_d = nc.dram_tensor("id16", [P, P], F16, kind="ExternalInput").ap()
    # output stays transposed [NOUT, B]; the host transposes for free
    out_d = nc.dram_tensor("out", [NOUT, B], F32, kind="ExternalOutput").ap()

    with tile.TileContext(nc) as tc, ExitStack() as ctx:
        const = ctx.enter_context(tc.tile_pool(name="const", bufs=1))
        imp = ctx.enter_context(tc.tile_pool(name="imp", bufs=1))
        zsp = ctx.enter_context(tc.tile_pool(name="zsp", bufs=4))
        msp = ctx.enter_context(tc.tile_pool(name="msp", bufs=6))
        upp = ctx.enter_context(tc.tile_pool(name="upp", bufs=2))
        mtp = ctx.enter_context(tc.tile_pool(name="mtp", bufs=2))
        ttp = ctx.enter_context(tc.tile_pool(name="ttp", bufs=1))

        wt = const.tile([KP, NF], F16, tag="wt")
        sfc = const.tile([P, 7 * NOUT], F16, tag="sfc")
        fcb = const.tile([NOUT, 1], F32, tag="fcb")
        id16 = const.tile([P, P], F16, tag="id16")
        ob = const.tile([16, B], F32, tag="ob")
        ims = [imp.tile([KP, HO * P], F16, tag=f"im{bt}", name=f"im{bt}")
               for bt in range(BT)]

        # ---- DMA issue schedule. Measured queue behavior: descriptor
        # processing is ~92ns/row regardless of row size, so full-tile
        # transfers (2-3KB rows) are 2-3x faster per byte than column
        # chunks; the sync hw queue starts ~8.8us, the scalar hw queue
        # ~10.9us but with priority over sync; gpsimd's software queue
        # spreads wide across engines. wt rides the scalar queue as ONE
        # full-tile transfer, and a small duplicate of its first 512
        # columns rides gpsimd so the A-chunk matmuls can start early.
        nc.sync.dma_start(ims[0][:, 0:512], im_d[0][:, 0:512])
        nc.sync.dma_start(ims[0][:, 512:1024], im_d[0][:, 512:1024])
        nc.sync.dma_start(ims[3][:], im_d[3])
        nc.scalar.dma_start(wt[:], wt_d)
        nc.scalar.dma_start(ims[0][:, 1024:1536], im_d[0][:, 1024:1536])
        nc.scalar.dma_start(ims[1][:], im_d[1])
        nc.scalar.dma_start(ims[2][:], im_d[2])
        nc.gpsimd.dma_start(sfc[:], sfc_d)
        nc.gpsimd.dma_start(fcb[:], fcb_d)
        nc.gpsimd.dma_start(id16[:], id16_d)

        # bt-major transposed ternary: bt block = [j(7) x 128 batch cols]
        tTall = ttp.tile([P, BT * 7 * P], F16, tag="tTall")

        with tc.tile_pool(name="zp", bufs=3, space="PSUM") as zp, \
             tc.tile_pool(name="ztp", bufs=1, space="PSUM") as ztp, \
             tc.tile_pool(name="pop", bufs=1, space="PSUM") as pop:

            def emit_fc_acc(bt):
                """FC matmuls + bias-activation for bt (acc rotates through
                the po PSUM tag/bank)."""
                accb = pop.tile([NOUT, P], F32, tag="po", name=f"acc{bt}")
                for j, kj in enumerate(FCJ):
                    nc.tensor.matmul(
                        accb[:, :],
                        lhsT=sfc[0:kj, j * NOUT:(j + 1) * NOUT],
                        rhs=tTall[0:kj, bt * 7 * P + j * P:
                                  bt * 7 * P + (j + 1) * P],
                        start=(j == 0), stop=(j == 6))
                nc.scalar.activation(ob[0:NOUT, bt * P:(bt + 1) * P],
                                     accb[:, :], ACT.Identity,
                                     bias=fcb[0:NOUT, 0:1], scale=1.0)

            def emit_fc_out(bt):
                """DMA the fp32 FC rows out (still transposed [10, 128])."""
                nc.sync.dma_start(out_d[:, bt * P:(bt + 1) * P],
                                  ob[0:NOUT, bt * P:(bt + 1) * P])

            def emit_finalize_block(fbt, fup3):
                mt = mtp.tile([P, FP], F16, tag="mt", name=f"mt{fbt}")
                lt = mtp.tile([P, FP], F16, tag="lt", name=f"lt{fbt}")
                t_ = mtp.tile([P, FP], F16, tag="t_", name=f"t{fbt}")
                _finalize(nc, fup3, mt, lt, t_, 0, FP)
                return t_

            def emit_transposes_copy(fbt, t_):
                po = pop.tile([P, 7 * P], F16, tag="po", name=f"po{fbt}")
                for j, kj in enumerate(FCJ):
                    nc.tensor.transpose(po[0:kj, j * P:(j + 1) * P],
                                        t_[:, j * 128:j * 128 + kj],
                                        id16[:])
                nc.scalar.copy(
                    tTall[:, fbt * 7 * P:(fbt + 1) * 7 * P], po[:, :])

            pend = None
            for bt in range(BT):
                # up3[p, half, h3*288+cg] - same layout as a flat half-major
                # [P, 1728] tile, 3D-viewed for the batched w-pool writes
                up3 = upp.tile([P, 2, 3 * 288], F16, tag="up", name=f"up{bt}")
                mt = lt = t_ = None
                po = None
                for h3 in range(3):
                    # m2[p, half, 1152]: both half-pairs of this h3 group
                    m2 = msp.tile([P, 2, NF], F16, tag="m2",
                                  name=f"m2{bt}_{h3}")
                    # zt4[p, half, 2*128]: all four tail chunks of the h3
                    # group share one PSUM bank, converted by ONE scalar op
                    zt4 = ztp.tile([P, 2, 256], F32, tag="zt",
                                   name=f"zt{bt}_{h3}")
                    for half in range(2):
                        pi = h3 * 2 + half
                        zh = []
                        for hh in range(2):
                            h = 4 * h3 + 2 * half + hh
                            k = h * P
                            z = zp.tile([P, 1024], F32, tag="z",
                                        name=f"z{bt}_{h}")
                            for n0, n1 in ((0, 512), (512, 1024)):
                                nc.tensor.matmul(
                                    z[:, n0:n1],
                                    lhsT=ims[bt][:, k:k + P],
                                    rhs=wt[:, n0:n1],
                                    start=True, stop=True)
                            nc.tensor.matmul(
                                zt4[:, half, hh * 128:(hh + 1) * 128],
                                lhsT=ims[bt][:, k:k + P],
                                rhs=wt[:, 1024:NF],
                                start=True, stop=True)
                            zh.append(z)
                        eng = PAIR_ENG[bt][pi]
                        if eng == 'S':
                            zs = zsp.tile([P, 2048], F16, tag="zs",
                                          name=f"zs{bt}_{pi}")
                            nc.scalar.copy(zs[:, 0:1024], zh[0][:, 0:1024])
                            nc.scalar.copy(zs[:, 1024:2048], zh[1][:, 0:1024])
                            nc.vector.tensor_max(m2[:, half, 0:1024],
                                                 zs[:, 0:1024],
                                                 zs[:, 1024:2048])
                        else:  # H: scalar converts h0, vector TTs psum h1
                            m0 = msp.tile([P, 1024], F16, tag="m0",
                                          name=f"m0{bt}_{pi}")
                            nc.scalar.copy(m0[:, 0:1024], zh[0][:, 0:1024])
                            nc.vector.tensor_max(m2[:, half, 0:1024],
                                                 zh[1][:, 0:1024],
                                                 m0[:, 0:1024])
                        # deferred pipeline of the previous bt, spread over
                        # this bt's first four pairs so the vector/PE queues
                        # never see a burst at the bt boundary:
                        #   (0,0) finalize  (0,1) transposes+copy
                        #   (1,0) FC acc    (1,1) FC out
                        if pend is not None:
                            if h3 == 0 and half == 0:
                                pend['t_'] = emit_finalize_block(
                                    pend['bt'], pend['up3'])
                            elif h3 == 0 and half == 1:
                                emit_transposes_copy(pend['bt'], pend['t_'])
                            elif h3 == 1 and half == 0:
                                emit_fc_acc(pend['bt'])
                            elif h3 == 1 and half == 1:
                                emit_fc_out(pend['bt'])
                                pend = None
                    # group tail: one scalar convert of all 4 tail chunks,
                    # one 3D-strided vector max into both halves' m2 tails
                    ztf = zsp.tile([P, 2, 256], F16, tag="ztf",
                                   name=f"ztf{bt}_{h3}")
                    nc.scalar.copy(ztf[:, :, :], zt4[:, :, :])
                    nc.vector.tensor_max(m2[:, :, 1024:NF],
                                         ztf[:, :, 0:128], ztf[:, :, 128:256])
                    # batched w-pool for both halves of this h3 group:
                    # level 1: [P,2,576], level 2: [P,2,288] into up slots
                    n4 = msp.tile([P, 2, 576], F16, tag="n4",
                                  name=f"n4{bt}_{h3}")
                    nc.vector.tensor_max(n4[:, :, :], m2[:, :, 0:576],
                                         m2[:, :, 576:NF])
                    nc.vector.tensor_max(up3[:, :, h3 * 288:(h3 + 1) * 288],
                                         n4[:, :, 0:288], n4[:, :, 288:576])

                    # bt3: finalize h3 groups early to shorten the tail
                    if bt == BT - 1 and h3 >= 1:
                        if h3 == 1:
                            c0, c1 = 0, 576
                            jlist = [0, 1, 2, 3]
                            mt = mtp.tile([P, FP], F16, tag="mt",
                                          name=f"mt{bt}")
                            lt = mtp.tile([P, FP], F16, tag="lt",
                                          name=f"lt{bt}")
                            t_ = mtp.tile([P, FP], F16, tag="t_",
                                          name=f"t{bt}")
                        else:
                            c0, c1 = 576, 864
                            jlist = [4, 5, 6]
                        _finalize(nc, up3, mt, lt, t_, c0, c1)
                        if po is None:
                            po = pop.tile([P, 7 * P], F16, tag="po",
                                          name=f"po{bt}")
                        for j in jlist:
                            kj = FCJ[j]
                            nc.tensor.transpose(
                                po[0:kj, j * P:(j + 1) * P],
                                t_[:, j * 128:j * 128 + kj], id16[:])
                        ccol = (jlist[0] * P, (jlist[-1] + 1) * P)
                        nc.scalar.copy(
                            tTall[:, bt * 7 * P + ccol[0]:
                                  bt * 7 * P + ccol[1]],
                            po[:, ccol[0]:ccol[1]])

                if bt < BT - 1:
                    pend = {'bt': bt, 'up3': up3}

            # bt3's FC runs in the tail
            emit_fc_acc(BT - 1)
            emit_fc_out(BT - 1)

    nc.compile()
    return nc


def _finalize(nc, up3, mt, lt, t_, c0, c1):
    """M = max over halves; lt = M < -1; t = (M > 0) - lt, on cols [c0,c1)."""
    nc.vector.tensor_max(mt[:, c0:c1], up3[:, 0, c0:c1], up3[:, 1, c0:c1])
    nc.vector.tensor_scalar(lt[:, c0:c1], mt[:, c0:c1], -1.0, None,
                            ALU.is_lt)
    nc.vector.scalar_tensor_tensor(
        t_[:, c0:c1], mt[:, c0:c1], 0.0, lt[:, c0:c1],
        ALU.is_gt, ALU.subtract)


_NC_CACHE = None


def kernel(x, conv_w, conv_b, bn_gamma, bn_beta, bn_mean, bn_var, fc_w, fc_b):
    global _NC_CACHE
    x = np.asarray(x, np.float32).reshape(BFULL, H * W)
    wt, sfc, fcb, eye16 = _host_prep(
        np.asarray(conv_w, np.float32), np.asarray(conv_b, np.float32),
        np.asarray(bn_gamma, np.float32), np.asarray(bn_beta, np.float32),
        np.asarray(bn_mean, np.float32), np.asarray(bn_var, np.float32),
        np.asarray(fc_w, np.float32), np.asarray(fc_b, np.float32))
    ims = _host_im2col(x)

    if _NC_CACHE is None:
        _NC_CACHE = _build()
    nc = _NC_CACHE

    in_maps = [
        dict(wt=wt, sfc=sfc, fcb=fcb, id16=eye16,
             **{f"im{bt}": ims[i][bt] for bt in range(BT)})
        for i in range(NCORES)
    ]
    res = run_bass_kernel_spmd(nc, in_maps, core_ids=list(range(NCORES)))
    out = np.concatenate([res.results[i]["out"].T for i in range(NCORES)],
                         axis=0)
    return out.astype(np.float32)
